# revision 56
# baseline (speedup 1.0000x reference)
"""Distributed Trainium2 Bass kernel for fused LayerNorm + causal multi-head
attention + output projection (B=2, T=2048, DIM=1024, H=16, D=64) on 8 cores.

Architecture (vs the AllGather baseline):
  - x is fed fully replicated to every core, host-pre-transposed to
    [dim, tok] bf16 layout. No AllGather of activations at all.
  - LN stats (mean / E[x^2]) for ALL 4096 tokens are computed locally per
    core with masked-ones column-sum matmuls ([8,512] PSUM rows, row c =
    token chunk c); mean subtraction and bias are folded into the QKV PSUM
    accumulation via an expanded-selector rank-2 matmul (corrx/stats_stack),
    so no DRAM bounce or row-flattening is needed.
  - Per-token rstd is applied three ways: Q at PSUM eviction via a
    selector-broadcast tile; K inside the exp (per-partition activation
    scale, scores partition = key token); V inside the vnat transpose-copy
    (per-partition tensor_scalar).
  - Attention: both heads' score tiles share one [128,1024] PSUM tile so
    exp runs fused; causal mask via one fused affine_select; PV uses the
    ones-column denominator trick.
  - A tiny warmup AllGather at t=0 absorbs the ~60us collective cold-start.
  - DMA "churn" + PE dummy matmuls keep the HAM/DVFS clock high through
    attention and the tail AllToAll (the A2A runs 2x faster at high clock).

All-bf16 matmuls (fp8 e4m3 dot products carry ~5% relative error regardless
of contraction length - fatal for the 2e-2 gate).
"""
import os
import sys
import types
import numpy as np
import ml_dtypes

# ---------------------------------------------------------------- constants
B, T, DIM, D = 2, 2048, 1024, 64
H = DIM // D            # 16 heads
NC = 8                  # cores
TOK = B * T             # 4096 tokens
TPC = TOK // NC         # 512 tokens per core
KT8 = DIM // 128        # 8 contraction tiles of 128
EPS = 1e-5
WS = 32.0               # host weight prescale (folded back at eviction)

TRACE = bool(int(os.environ.get("BASS_KERNEL_TRACE", "0")))
DUMMY_WARM = int(os.environ.get("DUMMY_WARM", "10"))
DUMMY_A2A = int(os.environ.get("DUMMY_A2A", "130"))

BF16_NP = ml_dtypes.bfloat16
F8_NP = ml_dtypes.float8_e4m3


def _ensure_ntff_hook():
    """The agent image lacks antenv.axon_hooks; recreate it so trace=True works."""
    if "antenv.axon_hooks" not in sys.modules:
        mod = types.ModuleType("antenv.axon_hooks")
        mod._hook = None
        def set_axon_ntff_profile_hook(h):
            mod._hook = h
        def get_axon_ntff_profile_hook():
            return mod._hook
        mod.set_axon_ntff_profile_hook = set_axon_ntff_profile_hook
        mod.get_axon_ntff_profile_hook = get_axon_ntff_profile_hook
        sys.modules["antenv.axon_hooks"] = mod
    m = sys.modules["antenv.axon_hooks"]
    if m.get_axon_ntff_profile_hook() is None:
        try:
            from trn_agent_boot.trn_boot import _ntff_profile_via_ctypes
            m.set_axon_ntff_profile_hook(
                _ntff_profile_via_ctypes("/opt/axon/libaxon_pjrt.so"))
        except Exception:
            pass


def build_graph():
    import concourse.bass as bass
    import concourse.bacc as bacc
    import concourse.tile as tile
    import concourse.mybir as mybir

    dt = mybir.dt
    F32, BF16, F8 = dt.float32, dt.bfloat16, dt.float8e4
    AF = mybir.ActivationFunctionType
    ALU = mybir.AluOpType
    PM = mybir.MatmulPerfMode.DoubleRow
    RG = [list(range(NC))]

    nc = bacc.Bacc(None, target_bir_lowering=False, debug=False, num_devices=NC)

    # ------------------------------------------------------------ I/O
    # x, host-transposed+tiled: [p, (chunk, k, t)] = x[512c+t, 128k+p], fp8
    xT_in = nc.dram_tensor("xT16", [128, 8 * KT8 * 512], BF16, kind="ExternalInput")
    # qkv weights, k-major: [p, (k, 384)] = 32*Wc[row, 128k+p], fp8
    wt_in = nc.dram_tensor("wt16", [128, KT8 * 384], BF16, kind="ExternalInput")
    # expanded rank-1 corrections: [16, (c, g, 128)]: row c -> W1, row 8+c -> b
    corr_in = nc.dram_tensor("corrx", [40, 8 * 3 * 128], BF16, kind="ExternalInput")
    # broadcast selector: [8, (c, 128)]: row c of block c = 1
    selbc_in = nc.dram_tensor("selbc", [8, 1024], BF16, kind="ExternalInput")
    # proj weights: [p, (k, 1024)] = 32*proj_w[o, 128k+p], fp8
    pwt_in = nc.dram_tensor("pwt16", [128, KT8 * DIM], BF16, kind="ExternalInput")
    pb_in = nc.dram_tensor("pb16", [1, DIM], BF16, kind="ExternalInput")
    idn_in = nc.dram_tensor("idn", [128, 128], BF16, kind="ExternalInput")
    ones_in = nc.dram_tensor("ones_r", [1, 128], BF16, kind="ExternalInput")
    emat_in = nc.dram_tensor("emat", [33, 128], BF16, kind="ExternalInput")
    # masked ones for column-sum rows: [p, (c, 8)]: 1 iff m == c, bf16
    emask_in = nc.dram_tensor("emask16", [128, 64], BF16, kind="ExternalInput")
    out_dram = nc.dram_tensor("out_c", [TPC, DIM], F32, kind="ExternalOutput")

    with tile.TileContext(nc) as tc:
        with (
            tc.tile_pool(name="persist", bufs=1) as pers,
            tc.tile_pool(name="dram", bufs=1, space="DRAM") as dram,
        ):
            # ---------------- DRAM bounce buffers ----------------
            a2a_in = dram.tile([NC * 128, TPC], BF16)
            a2a_out = dram.tile([NC * 128, TPC], BF16)
            warm_in = dram.tile([1, 16], BF16)
            warm_out = dram.tile([NC, 16], BF16, addr_space="Shared")

            # -------- warmup collective at t=0: absorbs cold-start so the
            # tail AllToAll begins in ~1us
            warm_sb = pers.tile([1, 16], BF16)
            nc.vector.memset(warm_sb[:], 0.0)
            nc.sync.dma_start(warm_in[:], warm_sb[:])
            nc.gpsimd.collective_compute(
                "AllGather", ALU.bypass, replica_groups=RG,
                ins=[warm_in[:].opt()], outs=[warm_out[:].opt()],
            )

            # ---------------- small constant loads (idn first) ----------
            idn_sb = pers.tile([128, 128], BF16)
            nc.sync.dma_start(idn_sb[:], idn_in[:])
            # first x chunk early: colsums can start before the weights land
            xT_sb = pers.tile([128, 8 * KT8 * 512], BF16)
            nc.sync.dma_start(xT_sb[:, 0:4096], xT_in[:, 0:4096])
            ones_sb = pers.tile([1, 128], BF16)
            nc.sync.dma_start(ones_sb[:], ones_in[:])
            emat_sb = pers.tile([33, 128], BF16)
            nc.sync.dma_start(emat_sb[:], emat_in[:])
            emask_sb = pers.tile([128, 64], BF16)
            nc.sync.dma_start(emask_sb[:], emask_in[:])
            corr_sb = pers.tile([40, 8 * 3 * 128], BF16)
            nc.sync.dma_start(corr_sb[:], corr_in[:])
            selbc_sb = pers.tile([8, 1024], BF16)
            nc.sync.dma_start(selbc_sb[:], selbc_in[:])
            wt_sb = pers.tile([128, KT8 * 384], BF16)
            nc.sync.dma_start(wt_sb[:], wt_in[:])
            pb_sb = pers.tile([1, DIM], BF16)
            nc.sync.dma_start(pb_sb[:], pb_in[:])

            # ---------------- x (bf16), remaining chunks ----------------
            for c in range(1, 8):
                nc.sync.dma_start(
                    xT_sb[:, 4096 * c:4096 * (c + 1)],
                    xT_in[:, 4096 * c:4096 * (c + 1)])

            # keep PE warm from the start (idn arrives first)
            with tc.tile_pool(name="ps_w", bufs=1, space="PSUM") as psw:
                dps = psw.tile([128, 128], F32, tag="dw")
                for i in range(DUMMY_WARM):
                    nc.tensor.matmul(dps[:], idn_sb[:], idn_sb[:],
                                     start=True, stop=True)

            def xck(c, j):
                """xT chunk c, k-subtile pair j: [128, 2, 512] fp8."""
                return xT_sb[:, 4096 * c:4096 * (c + 1)].rearrange(
                    "p (k t) -> p k t", t=512)[:, 2 * j:2 * j + 2, :]

            # ================= P1: LN stats for ALL tokens, locally =====
            # squares (engine-split), then masked-colsum matmuls into
            # [8, 512] PSUM rows (row c = token chunk c), then stats math.
            sq_engines = [nc.vector, nc.vector, nc.vector, nc.vector,
                          nc.scalar, nc.scalar,
                          nc.gpsimd, nc.gpsimd]
            mu_sb = pers.tile([8, 512], F32)
            vareps = pers.tile([8, 512], F32)
            std_f32 = pers.tile([8, 512], F32)
            rstd_bf = pers.tile([8, 512], BF16)
            stats_stack = pers.tile([40, 512], BF16)  # 0:8 = -mu, 32:40 = std
            nc.vector.memset(stats_stack[:], 0.0)

            emaskr = emask_sb[:].rearrange("p (c m) -> p c m", m=8)
            with (
                tc.tile_pool(name="xsq", bufs=2) as xsqp,
                tc.tile_pool(name="ps_st", bufs=1, space="PSUM") as psst,
            ):
                mean_ps = psst.tile([8, 512], F32, tag="mean")
                sq_ps = psst.tile([8, 512], F32, tag="sq")
                nmm = 8 * 8
                i = 0
                for c in range(8):
                    # mean colsums straight off the freshly-DMA'd chunk;
                    # masked stationary lands chunk c's sums on psum row c
                    for k in range(KT8):
                        nc.tensor.matmul(
                            mean_ps[:], emaskr[:, c],
                            xT_sb[:, 4096 * c + 512 * k:4096 * c + 512 * (k + 1)],
                            start=(i == 0), stop=(i == nmm - 1))
                        i += 1
                i = 0
                for c in range(8):
                    xsq = xsqp.tile([128, 4096], BF16, tag="xsq")
                    for k in range(KT8):
                        eng = sq_engines[k]
                        if eng is nc.scalar:
                            eng.activation(
                                xsq[:, 512 * k:512 * (k + 1)],
                                xT_sb[:, 4096 * c + 512 * k:
                                      4096 * c + 512 * (k + 1)],
                                AF.Square)
                        else:
                            eng.tensor_mul(
                                xsq[:, 512 * k:512 * (k + 1)],
                                xT_sb[:, 4096 * c + 512 * k:
                                      4096 * c + 512 * (k + 1)],
                                xT_sb[:, 4096 * c + 512 * k:
                                      4096 * c + 512 * (k + 1)])
                    for k in range(KT8):
                        nc.tensor.matmul(
                            sq_ps[:], emaskr[:, c],
                            xsq[:, 512 * k:512 * (k + 1)],
                            start=(i == 0), stop=(i == nmm - 1))
                        i += 1

                # stats math, batched over [8, 512] (rows 0..7 of the psums)
                nc.vector.tensor_scalar_mul(mu_sb[:], mean_ps[:], 1.0 / DIM)
                nc.vector.tensor_scalar(vareps[:], sq_ps[:], 1.0 / DIM, EPS,
                                        op0=ALU.mult, op1=ALU.add)
            nc.vector.tensor_mul(std_f32[:], mu_sb[:], mu_sb[:])
            nc.vector.tensor_sub(vareps[:], vareps[:], std_f32[:])
            nc.scalar.activation(std_f32[:], vareps[:], AF.Sqrt)
            nc.vector.reciprocal(vareps[:], std_f32[:])
            nc.vector.tensor_copy(rstd_bf[:], vareps[:])
            nc.vector.tensor_scalar_mul(stats_stack[0:8, :], mu_sb[:], -1.0)
            nc.vector.tensor_copy(stats_stack[32:40, :], std_f32[:])
            # rstd in token-partition layout [p, col] = rstd[token 128*col + p]
            # (feeds the exp per-partition scale and the vnat V-scaling)
            # token-partition rstd [p, col] = rstd[token 128*col + p], via PE
            # transposes of [8, 128] slices (col = 4c + s)
            rstd_tok = pers.tile([128, 32], F32)
            with tc.tile_pool(name="ps_rt", bufs=2, space="PSUM") as psrt:
                for s in range(4):
                    trs = psrt.tile([128, 8], BF16, tag="rt")
                    nc.tensor.transpose(
                        trs[:], rstd_bf[:, 128 * s:128 * (s + 1)],
                        idn_sb[0:8, 0:8])
                    nc.vector.tensor_copy(
                        rstd_tok[:].rearrange("p (c s) -> p c s", s=4)[:, :, s],
                        trs[:])

            # ================= P2: QKV (fp8 DoubleRow) ==================
            qkvT = []
            for name in ("qT", "kT", "vT"):
                qkvT.append(pers.tile([128, TOK], BF16, name=name))
            qT_sb, kT_sb, vT_sb = qkvT

            wtr = wt_sb[:].rearrange("p (k o) -> p k o", o=384)

            with (
                tc.tile_pool(name="ps_rb", bufs=2, space="PSUM") as psrb,
                tc.tile_pool(name="rb_sb", bufs=2) as rbsp,
                tc.tile_pool(name="ps_qkv", bufs=3, space="PSUM") as psq,
            ):
                for c in range(8):
                    rbc = psrb.tile([128, 512], F32, tag="rb")
                    nc.tensor.matmul(rbc[:], selbc_sb[:, 128 * c:128 * (c + 1)],
                                     rstd_bf[:], start=True, stop=True)
                    rb_sb = rbsp.tile([128, 512], BF16, tag="rbs")
                    nc.scalar.activation(rb_sb[:], rbc[:], AF.Identity)
                    for g in range(3):
                        psg = psq.tile([128, 512], F32, tag="qkv")
                        for k in range(KT8):
                            nc.tensor.matmul(
                                psg[:],
                                wtr[:, k, 128 * g:128 * (g + 1)],
                                xT_sb[:, 4096 * c + 512 * k:
                                      4096 * c + 512 * (k + 1)],
                                start=(k == 0), stop=False)
                        nc.tensor.matmul(
                            psg[:],
                            corr_sb[:, (3 * c + g) * 128:(3 * c + g + 1) * 128],
                            stats_stack[:], start=False, stop=True)
                        if g == 0:
                            # Q: per-token rstd/32 via broadcast tile (DVE)
                            nc.vector.tensor_mul(
                                qkvT[g][:, 512 * c:512 * (c + 1)],
                                psg[:], rb_sb[:])
                        else:
                            # K/V: evict raw/32; rstd is applied later via
                            # the exp per-partition scale (K) and the vnat
                            # transpose-copy scaling (V)
                            nc.scalar.activation(
                                qkvT[g][:, 512 * c:512 * (c + 1)],
                                psg[:], AF.Identity)

            # ================= P3/P4: V layout + attention ==============
            vnat = []
            for b in range(B):
                vb = pers.tile([128, 16 * 130], BF16, name=f"vnat{b}")
                nc.vector.memset(
                    vb[:].rearrange("p (j a w) -> p j a w", a=2, w=65)[:, :, :, 64:65], 1.0)
                vnat.append(vb)
            attnT = pers.tile([128, TOK], BF16)
            sums_col = pers.tile([33, 512], F32)
            nc.vector.memset(sums_col[:], 1.0)

            with (
                tc.tile_pool(name="pt", bufs=4) as ptp,
                tc.tile_pool(name="ps_s", bufs=2, space="PSUM") as pss,
                tc.tile_pool(name="ps_pv", bufs=3, space="PSUM") as psp,
                tc.tile_pool(name="ps_bc", bufs=1, space="PSUM") as psb,
                tc.tile_pool(name="sm", bufs=2) as smp,
            ):
                def emit_vtr(b, j):
                    # vtr partitions = key tokens: scale by rstd there
                    col = 16 * b + j
                    vtr = psb.tile([128, 128], BF16, tag="bc")
                    nc.tensor.transpose(
                        vtr[:],
                        vT_sb[:, b * T + 128 * j: b * T + 128 * (j + 1)],
                        idn_sb[:])
                    nc.vector.tensor_scalar_mul(
                        vnat[b][:, 130 * j: 130 * j + 64], vtr[:, 0:64],
                        rstd_tok[:, col:col + 1])
                    nc.vector.tensor_scalar_mul(
                        vnat[b][:, 130 * j + 65: 130 * j + 129], vtr[:, 64:128],
                        rstd_tok[:, col:col + 1])

                def emit_attention(b, inject):
                    ii = 0
                    for qc in range(4):
                        q0 = b * T + 512 * qc
                        pvA = psp.tile([65, 512], F32, tag="pv")
                        pvB = psp.tile([65, 512], F32, tag="pv")
                        nkp = 4 * qc + 4
                        pend = None
                        for kp in range(nkp):
                            k0 = b * T + 128 * kp
                            # columns below 128*d are fully causal-masked for
                            # diagonal tiles: skip them in scores/PV (stale
                            # PSUM there is zero-filled by affine_select)
                            lo = 128 * max(0, kp - 4 * qc)
                            sAB = pss.tile([128, 1024], F32, tag="s")
                            nc.tensor.matmul(sAB[:, lo:512],
                                             kT_sb[0:64, k0:k0 + 128],
                                             qT_sb[0:64, q0 + lo:q0 + 512],
                                             start=True, stop=True)
                            nc.tensor.matmul(sAB[:, 512 + lo:1024],
                                             kT_sb[64:128, k0:k0 + 128],
                                             qT_sb[64:128, q0 + lo:q0 + 512],
                                             start=True, stop=True)
                            if pend is not None:
                                pkp, pp = pend
                                plo = 128 * max(0, pkp - 4 * qc)
                                nc.tensor.matmul(pvA[:, plo:512],
                                                 vnat[b][:, 130 * pkp:130 * pkp + 65],
                                                 pp[:, plo:512],
                                                 start=(pkp == 0), stop=False,
                                                 skip_group_check=True)
                                nc.tensor.matmul(pvB[:, plo:512],
                                                 vnat[b][:, 130 * pkp + 65:130 * pkp + 130],
                                                 pp[:, 512 + plo:1024],
                                                 start=(pkp == 0), stop=False,
                                                 skip_group_check=True)
                            pAB = ptp.tile([128, 1024], BF16, tag="pAB")
                            # per-partition scale = rstd of the key tokens.
                            # one contiguous subrange [lo:1024] skips head-A's
                            # fully-masked columns at no extra instruction
                            # (affine_select zero-fills the stale region)
                            nc.scalar.activation(
                                pAB[:, lo:1024], sAB[:, lo:1024], AF.Exp,
                                scale=rstd_tok[:, 16 * b + kp:16 * b + kp + 1])
                            if kp >= 4 * qc:
                                base = 512 * qc - 128 * kp
                                nc.gpsimd.affine_select(
                                    pAB[:], pAB[:], pattern=[[0, 2], [1, 512]],
                                    compare_op=ALU.is_ge, fill=0.0,
                                    base=base, channel_multiplier=-1)
                            pend = (kp, pAB)
                            if ii < len(inject):
                                inject[ii]()
                                ii += 1
                        pkp, pp = pend
                        plo = 128 * max(0, pkp - 4 * qc)
                        nc.tensor.matmul(pvA[:, plo:512],
                                         vnat[b][:, 130 * pkp:130 * pkp + 65],
                                         pp[:, plo:512],
                                         start=(pkp == 0), stop=True,
                                         skip_group_check=True)
                        nc.tensor.matmul(pvB[:, plo:512],
                                         vnat[b][:, 130 * pkp + 65:130 * pkp + 130],
                                         pp[:, 512 + plo:1024],
                                         start=(pkp == 0), stop=True,
                                         skip_group_check=True)
                        nc.vector.tensor_copy(sums_col[0:1, :], pvA[64:65, :])
                        nc.vector.tensor_copy(sums_col[32:33, :], pvB[64:65, :])
                        rec = smp.tile([33, 512], F32, tag="rec")
                        nc.vector.reciprocal_approx_fast(rec[:], sums_col[:])
                        recb = smp.tile([33, 512], BF16, tag="recb")
                        nc.vector.tensor_copy(recb[:], rec[:])
                        bc2 = psb.tile([128, 512], F32, tag="bc")
                        nc.tensor.matmul(bc2[:], emat_sb[:], recb[:],
                                         start=True, stop=True)
                        bc2s = smp.tile([128, 512], BF16, tag="bc2s")
                        nc.vector.tensor_copy(bc2s[:], bc2[:])
                        nc.vector.tensor_tensor(
                            attnT[0:64, q0:q0 + 512], pvA[0:64, :],
                            bc2s[0:64, :], op=ALU.mult)
                        nc.vector.tensor_tensor(
                            attnT[64:128, q0:q0 + 512], pvB[0:64, :],
                            bc2s[64:128, :], op=ALU.mult)
                        r = 4 * b + qc
                        nc.sync.dma_start(a2a_in[128 * r:128 * (r + 1), :],
                                          attnT[:, TPC * r:TPC * (r + 1)])
                    while ii < len(inject):
                        inject[ii]()
                        ii += 1

                with tc.tile_pool(name="churn", bufs=4) as chp:
                    def emit_churn(i):
                        ct = chp.tile([128, 1024], BF16, tag="ch")
                        nc.sync.dma_start(
                            ct[:], xT_in[:, 4096 * (i % 8):4096 * (i % 8) + 1024])

                    def noop():
                        pass

                    for j in range(16):
                        emit_vtr(0, j)
                    # attn0 injects: vtr(1) first, then churn every other slot
                    inj0 = [(lambda jj=j: emit_vtr(1, jj)) for j in range(16)]
                    inj0 += [(lambda ii=i: emit_churn(ii)) if i % 2 == 0 else noop
                             for i in range(24)]
                    # attn1: churn every 3rd kp so activity spans to the end
                    inj1 = [(lambda ii=i: emit_churn(ii)) if i % 3 == 0 else noop
                            for i in range(40)]
                    emit_attention(0, inj0)
                    emit_attention(1, inj1)

            # ================= P5: AllToAll (slices staged per-qc above) ===
            nc.gpsimd.collective_compute(
                "AllToAll", ALU.bypass, replica_groups=RG,
                ins=[a2a_in[:].opt()], outs=[a2a_out[:].opt()],
            )

            # proj weights can stream any time before proj
            pwt_sb = pers.tile([128, KT8 * DIM], BF16)
            nc.sync.dma_start(pwt_sb[:], pwt_in[:])

            # keep the whole chip warm while the A2A is in flight: PE
            # dummies + DMA churn + DVE/ACT passes (HAM ramps on broad
            # activity, and the collective runs ~2x faster at high clock)
            with (
                tc.tile_pool(name="ps_dummy2", bufs=1, space="PSUM") as psd2,
                tc.tile_pool(name="churn2", bufs=4) as ch2,
                tc.tile_pool(name="vch", bufs=2) as vch,
            ):
                dps2 = psd2.tile([128, 512], F32, tag="d2")
                for i in range(DUMMY_A2A):
                    nc.tensor.matmul(dps2[:], idn_sb[:],
                                     qT_sb[:, 512 * (i % 8):512 * (i % 8) + 512],
                                     start=True, stop=True)
                    if i % 16 == 0:
                        ct = ch2.tile([128, 1024], BF16, tag="ch2")
                        nc.sync.dma_start(
                            ct[:], xT_in[:, 4096 * (i % 8):4096 * (i % 8) + 1024])
                    if i % 16 == 0:
                        vt = vch.tile([128, 2048], BF16, tag="vc")
                        nc.vector.tensor_copy(vt[:], qT_sb[:, 0:2048])
                        nc.scalar.activation(vt[:], qT_sb[:, 0:2048],
                                             AF.Identity)

            # ================= P6: output projection (fp8 DoubleRow) ======
            pwtr = pwt_sb[:].rearrange("p (k o) -> p k o", o=DIM)
            with (
                tc.tile_pool(name="projx", bufs=1) as pxp,
                tc.tile_pool(name="ps_o", bufs=3, space="PSUM") as pso,
                tc.tile_pool(name="outp", bufs=2) as outp,
            ):
                aT = pxp.tile([128, KT8 * TPC], BF16, tag="aT")
                for ck in range(KT8):
                    nc.sync.dma_start(aT[:, 512 * ck:512 * (ck + 1)],
                                      a2a_out[128 * ck:128 * (ck + 1), :])
                aTr = aT[:].rearrange("p (k t) -> p k t", t=TPC)
                ev = 0
                for tt in range(4):
                    ot = outp.tile([128, DIM], F32, tag="ot")
                    for half in range(2):
                        pso_t = pso.tile([128, 512], F32, tag="po")
                        for k in range(KT8):
                            nc.tensor.matmul(
                                pso_t[:],
                                aTr[:, k, 128 * tt:128 * (tt + 1)],
                                pwtr[:, k, 512 * half:512 * (half + 1)],
                                start=(k == 0), stop=False)
                        nc.tensor.matmul(
                            pso_t[:], ones_sb[:],
                            pb_sb[:, 512 * half:512 * (half + 1)],
                            start=False, stop=True)
                        if ev % 2 == 0:
                            nc.vector.tensor_copy(
                                ot[:, 512 * half:512 * (half + 1)], pso_t[:])
                        else:
                            nc.scalar.activation(
                                ot[:, 512 * half:512 * (half + 1)], pso_t[:],
                                AF.Identity)
                        ev += 1
                    nc.sync.dma_start(out_dram[128 * tt:128 * (tt + 1), :], ot[:])

    nc.compile()
    return nc


def host_prep(inputs):
    x = np.asarray(inputs["x"], np.float32).reshape(TOK, DIM)
    ln_w = np.asarray(inputs["ln_w"], np.float32)
    ln_b = np.asarray(inputs["ln_b"], np.float32)
    qkv_w = np.asarray(inputs["qkv_w"], np.float32)
    qkv_b = np.asarray(inputs["qkv_b"], np.float32)
    proj_w = np.asarray(inputs["proj_w"], np.float32)
    proj_b = np.asarray(inputs["proj_b"], np.float32)

    # fold LN affine into qkv weights; fold 1/sqrt(D) into Q rows
    Wp = qkv_w * ln_w[None, :]
    bp = qkv_b + qkv_w @ ln_b
    Wp[0:DIM] *= D ** -0.5
    bp[0:DIM] *= D ** -0.5

    # x, transposed + tiled: [p, (c, k, t)] = x[512c+t, 128k+p]
    xT16 = np.ascontiguousarray(
        x.T.reshape(KT8, 128, 8, 512).transpose(1, 2, 0, 3).reshape(128, -1)
    ).astype(BF16_NP)

    # proj weights: [p, (k, o)] = proj_w[o, 128k+p]
    pwt16 = np.ascontiguousarray(
        proj_w.T.reshape(KT8, 128, DIM).transpose(1, 0, 2).reshape(128, -1)
    ).astype(BF16_NP)
    pb16 = proj_b.reshape(1, DIM).astype(BF16_NP)

    idn = np.eye(128, dtype=np.float32).astype(BF16_NP)
    ones_r = np.ones((1, 128), BF16_NP)
    emat = np.zeros((33, 128), np.float32)
    emat[0, 0:64] = 1.0
    emat[32, 64:128] = 1.0
    emat = emat.astype(BF16_NP)
    emask = np.zeros((128, 8, 8), np.float32)
    for c in range(8):
        emask[:, c, c] = 1.0
    emask16 = emask.reshape(128, 64).astype(BF16_NP)

    in_maps = []
    for c in range(NC):
        rows = []
        for blk in range(3):
            for h in (2 * c, 2 * c + 1):
                rows.extend(range(blk * DIM + h * D, blk * DIM + (h + 1) * D))
        rows = np.array(rows)
        Wc = Wp[rows]                      # [384, 1024]
        bc = bp[rows]                      # [384]
        wt16 = np.ascontiguousarray(
            Wc.T.reshape(KT8, 128, 384).transpose(1, 0, 2).reshape(128, -1)
        ).astype(BF16_NP)
        W1 = Wc.sum(axis=1)                # [384]
        corrx = np.zeros((40, 8 * 3 * 128), np.float32)
        for cc in range(8):
            for g in range(3):
                blk = (3 * cc + g) * 128
                corrx[cc, blk:blk + 128] = W1[128 * g:128 * (g + 1)]
                corrx[32 + cc, blk:blk + 128] = bc[128 * g:128 * (g + 1)]
        selbc = np.zeros((8, 8, 128), np.float32)
        for cc in range(8):
            selbc[cc, cc, :] = 1.0
        in_maps.append(dict(
            xT16=xT16, wt16=wt16, corrx=corrx.astype(BF16_NP),
            selbc=selbc.reshape(8, 1024).astype(BF16_NP),
            pwt16=pwt16, pb16=pb16,
            idn=idn, ones_r=ones_r, emat=emat, emask16=emask16,
        ))
    return in_maps


_CACHED = {}


def kernel(**inputs) -> np.ndarray:
    _ensure_ntff_hook()
    from concourse import bass_utils
    if TRACE:
        bass_utils.upload_artifacts = lambda tmpdir: "/tmp/noupload"

    if "nc" not in _CACHED:
        _CACHED["nc"] = build_graph()
    nc = _CACHED["nc"]

    in_maps = host_prep(inputs)
    res = bass_utils.run_bass_kernel_spmd(
        nc, in_maps, core_ids=list(range(NC)), trace=TRACE,
        trace_cores=list(range(NC)) if TRACE else None)
    _CACHED["last_result"] = res
    out = np.concatenate([res.results[c]["out_c"] for c in range(NC)], axis=0)
    return out.reshape(B, T, DIM).astype(np.float32)
